# revision 1
# baseline (speedup 1.0000x reference)
"""BERT-CRF Viterbi decode kernel for Trainium2 (Bass/Tile), 8-core data parallel.

Full inputs in, full outputs out. Internally shards batch B=64 across 8 cores
(8 sequences each). Per core, with scan rows r = b*16 + c (c = chunk of 32
timesteps):

  Stage A (u-tiled, fused with scan phase 1):
    for each scan step u (0..31), load sentences for all 128 rows at local
    step u, transpose h-chunks on PE, batched matmul (lhsT = W^T chunk [128,4],
    rhs = 4 steps' transposed sentences [128,512]) -> emissions^T in PSUM,
    fix-transpose back to [rows, 4], write directly into the SBUF scan tile.
    Phase 1 (chunk transfer-matrix recurrence) consumes each step's emissions
    as they land, hidden under stage A's PE/DMA time.
  Phase 2: boundary scores across chunks (sequential over 16, rows 0..7).
  Phase 3: all scores from boundaries + stored prefix matrices (2 big ops).
  Phase 4: backpointer one-hots, first-argmax semantics (6 big ops).
  Phase 5: one-hot matrix backtracking (no gathers).
"""
import sys
for p in ("/opt/trn_rl_repo", "/root/.axon_site/_ro/trn_rl_repo"):
    if p not in sys.path:
        sys.path.append(p)

import numpy as np
import concourse.bass as bass
import concourse.tile as tile
from concourse import mybir
from concourse.bass_utils import run_bass_kernel_spmd

F32 = mybir.dt.float32
F32R = mybir.dt.float32r
I32 = mybir.dt.int32
AX = mybir.AxisListType
OP = mybir.AluOpType

B, T, H, K = 64, 512, 768, 4
NCORES = 8
BC = B // NCORES          # 8 sequences per core
C, L = 16, 32             # chunks per sequence, steps per chunk
ROWS = BC * C             # 128 partition rows
HCH = H // 128            # 6 h-chunks
UG = 4                    # steps per u-group (batched matmul width 4*128=512)

_NC_CACHE = {}


def build_nc():
    nc = bass.Bass()
    sent = nc.declare_dram_parameter("sentences", [BC, T, H], F32, isOutput=False)
    Wd = nc.declare_dram_parameter("W", [K, H], F32, isOutput=False)
    identd = nc.declare_dram_parameter("identc", [128, 128], F32, isOutput=False)
    # rowconsts[128, 64]: wfirst | iw | ident4 | end | ttr | tinit | bias
    rcd = nc.declare_dram_parameter("rowconsts", [128, 64], F32, isOutput=False)
    tagsd = nc.declare_dram_parameter("tags", [BC, T], I32, isOutput=True)

    with tile.TileContext(nc) as tc:
        with tc.tile_pool(name="singles", bufs=1) as singles, \
             tc.tile_pool(name="sent_pool", bufs=6) as sent_pool, \
             tc.tile_pool(name="st_pool", bufs=2) as st_pool, \
             tc.tile_pool(name="tmp_pool", bufs=2) as tmp_pool, \
             tc.tile_pool(name="ps_tr", bufs=3, space="PSUM") as ps_tr, \
             tc.tile_pool(name="ps_eT", bufs=2, space="PSUM") as ps_eT, \
             tc.tile_pool(name="ps_fix", bufs=2, space="PSUM") as ps_fix:

            # ---------- prefetch first sentence group ----------
            pre_sents = []
            for uu in range(UG):
                s_sb = sent_pool.tile([128, H], F32, tag="sent")
                src0 = bass.AP(
                    tensor=sent[:].tensor, offset=uu * H,
                    ap=[[T * H, BC], [L * H, C], [1, H]])
                nc.sync.dma_start(s_sb, src0)
                pre_sents.append(s_sb)

            # ---------- constants ----------
            ident = singles.tile([128, 128], F32)
            nc.sync.dma_start(ident, identd[:])
            rc = singles.tile([128, 64], F32)
            nc.sync.dma_start(rc, rcd[:])
            wfirst = rc[:, 0:4]
            iw4 = rc[:, 4:8]
            id4 = rc[:, 8:24]
            end_sb = rc[:, 24:28]
            ttr = rc[:, 28:44]
            tinit = rc[:, 44:60]
            bias4 = rc[:, 60:64]

            # ---------- W^T in SBUF: wt[p = h within chunk, ch, k] ----------
            w_raw = singles.tile([K, H], F32)
            nc.sync.dma_start(w_raw, Wd[:])
            wt_sb = singles.tile([128, HCH, K], F32)
            for ch in range(HCH):
                wt_ps = ps_fix.tile([128, K], F32, tag="fix")
                nc.tensor.transpose(wt_ps, w_raw[:, ch * 128:(ch + 1) * 128],
                                    ident[0:K, 0:K])
                nc.scalar.copy(wt_sb[:, ch, :], wt_ps)

            # scan emissions tile, written directly by stage A
            emsc = singles.tile([128, L * K], F32)
            emv = emsc.rearrange("p (u j) -> p u j", u=L)

            # phase-1 state: prefix transfer matrices Apre[row, u, i, j]
            Apre = singles.tile([128, L, 4, 4], F32)

            # views
            ttrT_v = ttr.rearrange("p (k j) -> p k j", k=4).transpose([0, 2, 1])  # [p,j,k] = trans[k,j]
            ttr_ji = ttr.rearrange("p (i j) -> p i j", i=4).transpose([0, 2, 1])  # [p,j,i] = trans[i,j]

            # ---------- Stage A (u-tiled) fused with phase 1 ----------
            sA = nc.named_scope("stageA")
            sA.__enter__()
            for g in range(L // UG):
                if g == 0:
                    sents = pre_sents
                else:
                    sents = []
                    for uu in range(UG):
                        u = g * UG + uu
                        s_sb = sent_pool.tile([128, H], F32, tag="sent")
                        # row (b*16+c) <- sentences[b, c*32 + u, :]
                        src = bass.AP(
                            tensor=sent[:].tensor, offset=u * H,
                            ap=[[T * H, BC], [L * H, C], [1, H]])
                        nc.sync.dma_start(s_sb, src)
                        sents.append(s_sb)
                # transposes: sT[p=h, ch, uu, rows]
                sT_sb = st_pool.tile([128, HCH, UG, 128], F32)
                for ch in range(HCH):
                    trp = ps_tr.tile([128, UG * 128], F32, tag="trps")
                    for uu in range(UG):
                        nc.tensor.transpose(
                            trp[:, uu * 128:(uu + 1) * 128],
                            sents[uu][:, ch * 128:(ch + 1) * 128],
                            ident)
                    nc.scalar.copy(
                        sT_sb[:, ch, :, :].rearrange("p a b -> p (a b)"), trp)
                # batched matmuls: out eT[k, uu*128+row] accum over ch
                eT_ps = ps_eT.tile([4, UG * 128], F32, tag="eT")
                for ch in range(HCH):
                    nc.tensor.matmul(
                        eT_ps, wt_sb[:, ch, :],
                        sT_sb[:, ch, :, :].rearrange("p a b -> p (a b)"),
                        start=(ch == 0), stop=(ch == HCH - 1))
                eT_sb = st_pool.tile([4, UG * 128], F32, tag="eTsb")
                nc.scalar.copy(eT_sb, eT_ps)
                # fix-transpose each uu back to [rows, 4] and land in emsc
                for uu in range(UG):
                    u = g * UG + uu
                    fx = ps_fix.tile([128, K], F32, tag="fix")
                    nc.tensor.transpose(
                        fx, eT_sb[:, uu * 128:(uu + 1) * 128], ident[0:K, 0:K])
                    nc.scalar.copy(emsc[:, u * 4:(u + 1) * 4], fx)
                # bias (reference adds b last): emsc[:, g] += b
                nc.vector.tensor_tensor(
                    emv[:, g * UG:(g + 1) * UG, :],
                    emv[:, g * UG:(g + 1) * UG, :],
                    bias4.unsqueeze(1).to_broadcast((128, UG, 4)),
                    OP.add)
                # ---- phase 1 steps for this group ----
                for uu in range(UG):
                    u = g * UG + uu
                    if u == 0:
                        nc.vector.tensor_tensor(
                            Apre[:, 0, :, :],
                            tinit.rearrange("p (i j) -> p i j", i=4),
                            emv[:, 0, :].unsqueeze(1).to_broadcast((128, 4, 4)),
                            OP.add)
                    else:
                        p1tmp = tmp_pool.tile([128, 4, 4, 4], F32, tag="p1tmp")
                        # tmp[i,j,k] = A[i,k] + trans[k,j]
                        nc.vector.tensor_tensor(
                            p1tmp,
                            Apre[:, u - 1, :, :].unsqueeze(2).to_broadcast((128, 4, 4, 4)),
                            ttrT_v.unsqueeze(1).to_broadcast((128, 4, 4, 4)),
                            OP.add)
                        p1red = tmp_pool.tile([128, 4, 4], F32, tag="p1red")
                        nc.vector.reduce_max(p1red, p1tmp, axis=AX.X)
                        nc.vector.tensor_tensor(
                            Apre[:, u, :, :], p1red,
                            emv[:, u, :].unsqueeze(1).to_broadcast((128, 4, 4)), OP.add)
            sA.__exit__(None, None, None)

            # regroup A_c = Apre[:, L-1] to by-b layout [8, C*16]
            _sp2 = nc.named_scope("p2")
            _sp2.__enter__()
            abyb = singles.tile([BC, C * 16], F32)
            nc.sync.dma_start(abyb, Apre[:, L - 1, :, :].rearrange("p a b -> p (a b)"))
            abv = abyb.rearrange("p (c i j) -> p c i j", c=C, i=4)

            # ----- phase 2: boundary scores sbound[8, (C+1)*4], slot0 = 0 -----
            sbound = singles.tile([BC, (C + 1) * 4], F32)
            nc.vector.memset(sbound[:, 0:4], 0.0)
            sbv = sbound.rearrange("p (c j) -> p c j", c=C + 1)
            for c in range(C):
                p2tmp = tmp_pool.tile([BC, 4, 4], F32, tag="p2tmp")
                # tmp[j,i] = s[i] + A_c[i,j]
                nc.vector.tensor_tensor(
                    p2tmp,
                    sbv[:, c, :].unsqueeze(1).to_broadcast((BC, 4, 4)),
                    abv[:, c, :, :].transpose([0, 2, 1]),
                    OP.add)
                nc.vector.reduce_max(sbv[:, c + 1, :], p2tmp, axis=AX.X)
            _sp2.__exit__(None, None, None)

            # ----- phase 3 (parallel): scores[128, (L+1)*4] from boundary + Apre -----
            _sp3 = nc.named_scope("p3")
            _sp3.__enter__()
            scores = singles.tile([128, (L + 1) * 4], F32)
            nc.sync.dma_start(scores[:, 0:4], sbound[:, 0:C * 4])
            scv = scores.rearrange("p (u i) -> p u i", u=L + 1)
            p3tmp = singles.tile([128, L, 4, 4], F32)   # [u, j, i]
            nc.vector.tensor_tensor(
                p3tmp,
                scores[:, 0:4].unsqueeze(1).unsqueeze(1).to_broadcast((128, L, 4, 4)),
                Apre.transpose([0, 1, 3, 2]),
                OP.add)
            nc.vector.reduce_max(scv[:, 1:, :], p3tmp, axis=AX.X)
            _sp3.__exit__(None, None, None)

            # ----- phase 4: backpointer one-hots Pall[128, L, j, i] -----
            _sp4 = nc.named_scope("p4")
            _sp4.__enter__()
            cand = singles.tile([128, L, 4, 4], F32)
            nc.vector.tensor_tensor(
                cand,
                scv[:, 0:L, :].unsqueeze(2).to_broadcast((128, L, 4, 4)),
                ttr_ji.unsqueeze(1).to_broadcast((128, L, 4, 4)),
                OP.add)
            mxP = tmp_pool.tile([128, L, 4], F32, tag="mxP")
            nc.vector.reduce_max(mxP, cand, axis=AX.X)
            eqP = singles.tile([128, L, 4, 4], F32)
            nc.vector.tensor_tensor(eqP, cand, mxP.unsqueeze(3).to_broadcast((128, L, 4, 4)), OP.is_equal)
            nc.vector.tensor_tensor(
                eqP, eqP,
                wfirst.unsqueeze(1).unsqueeze(1).to_broadcast((128, L, 4, 4)),
                OP.mult)
            nc.vector.reduce_max(mxP, eqP, axis=AX.X)
            Pall = singles.tile([128, L, 4, 4], F32)
            nc.vector.tensor_tensor(Pall, eqP, mxP.unsqueeze(3).to_broadcast((128, L, 4, 4)), OP.is_equal)
            _sp4.__exit__(None, None, None)

            # ----- best_last one-hot on rows 0..7 -----
            ebyb = singles.tile([BC, C * 4], F32)
            ebv = ebyb.rearrange("p (c j) -> p c j", c=C)
            fin = tmp_pool.tile([BC, 4], F32, tag="fin")
            nc.vector.tensor_add(fin, sbv[:, C, :], end_sb[0:BC, :])
            mxf = tmp_pool.tile([BC, 1], F32, tag="mxf")
            nc.vector.reduce_max(mxf, fin, axis=AX.X)
            eqf = tmp_pool.tile([BC, 4], F32, tag="eqf")
            nc.vector.tensor_tensor(eqf, fin, mxf.to_broadcast((BC, 4)), OP.is_equal)
            nc.vector.tensor_tensor(eqf, eqf, wfirst[0:BC, :], OP.mult)
            nc.vector.reduce_max(mxf, eqf, axis=AX.X)
            nc.vector.tensor_tensor(ebv[:, C - 1, :], eqf, mxf.to_broadcast((BC, 4)), OP.is_equal)

            # ----- phase 5b: suffix maps Sall[128, L, x, i] + Ofull -----
            _sp5b = nc.named_scope("p5b")
            _sp5b.__enter__()
            Sall = singles.tile([128, L, 4, 4], F32)
            nc.vector.tensor_copy(Sall[:, L - 1, :, :], id4.rearrange("p (x i) -> p x i", x=4))
            for u in range(L - 2, -2, -1):
                p5tmp = tmp_pool.tile([128, 4, 4, 4], F32, tag="p5tmp")
                # tmp[x,i,y] = S_{u+1}[x,y] * P_{u+1}[y,i]
                nc.vector.tensor_tensor(
                    p5tmp,
                    Sall[:, u + 1, :, :].unsqueeze(2).to_broadcast((128, 4, 4, 4)),
                    Pall[:, u + 1, :, :].transpose([0, 2, 1]).unsqueeze(1).to_broadcast((128, 4, 4, 4)),
                    OP.mult)
                if u >= 0:
                    nc.vector.reduce_sum(Sall[:, u, :, :], p5tmp, axis=AX.X)
                else:
                    Ofull = singles.tile([128, 16], F32)
                    nc.vector.reduce_sum(Ofull.rearrange("p (x i) -> p x i", x=4),
                                         p5tmp, axis=AX.X)
            _sp5b.__exit__(None, None, None)

            # regroup Ofull to by-b [8, C*16]
            _sp5c = nc.named_scope("p5c")
            _sp5c.__enter__()
            obyb = singles.tile([BC, C * 16], F32)
            nc.sync.dma_start(obyb, Ofull)

            obv = obyb.rearrange("p (c x i) -> p c x i", c=C, x=4)

            # ----- phase 5c: boundary tags backward -----
            for c in range(C - 1, 0, -1):
                p5ctmp = tmp_pool.tile([BC, 4, 4], F32, tag="p5ctmp")
                # tmp[i,x] = E_c[x] * Ofull_c[x,i]
                nc.vector.tensor_tensor(
                    p5ctmp,
                    ebv[:, c, :].unsqueeze(1).to_broadcast((BC, 4, 4)),
                    obv[:, c, :, :].transpose([0, 2, 1]),
                    OP.mult)
                nc.vector.reduce_sum(ebv[:, c - 1, :], p5ctmp, axis=AX.X)

            # broadcast E to rows: ebc[128, 4], row b*16+c = E_c[b]
            ebc = singles.tile([128, 4], F32)
            nc.sync.dma_start(ebc, ebyb)
            _sp5c.__exit__(None, None, None)

            # ----- phase 5d: tags -----
            _sp5d = nc.named_scope("p5d")
            _sp5d.__enter__()
            G = tmp_pool.tile([128, 4, 4], F32, tag="G")
            nc.vector.tensor_tensor(
                G,
                ebc.unsqueeze(2).to_broadcast((128, 4, 4)),
                iw4.unsqueeze(1).to_broadcast((128, 4, 4)),
                OP.mult)
            p5dtmp = singles.tile([128, L, 4, 4], F32)
            nc.vector.tensor_tensor(
                p5dtmp, Sall,
                G.unsqueeze(1).to_broadcast((128, L, 4, 4)),
                OP.mult)
            tagf = tmp_pool.tile([128, L], F32, tag="tagf")
            nc.vector.reduce_sum(tagf, p5dtmp.rearrange("p u x i -> p u (x i)"), axis=AX.X)
            tagi = tmp_pool.tile([128, L], I32, tag="tagi")
            nc.vector.tensor_copy(tagi, tagf)
            nc.sync.dma_start(tagsd[:].rearrange("b (c t) -> b c t", c=C), tagi)
            _sp5d.__exit__(None, None, None)

    return nc


def _split_multi_waits(nc):
    """Walrus (bass2jax path) allows very few embedded sync waits per
    instruction (PE matmul: exactly 1). Hoist multi-waits onto standalone
    single-wait InstDrain instructions on the same engine, preserving order."""
    for f in nc.m.functions:
        for blk in f.blocks:
            insts = blk.instructions
            i = 0
            while i < len(insts):
                ins = insts[i]
                si = ins.sync_info
                w = list(si.on_wait) if (si is not None and si.on_wait) else []
                if len(w) >= 2:
                    for k, wait in enumerate(w):
                        d = mybir.InstEventSemaphore(
                            name=nc.get_next_instruction_name(), ins=[], outs=[])
                        d.engine = ins.engine
                        d.sync_info = mybir.SyncInfo(on_wait=[wait], on_update=[])
                        insts.insert(i + k, d)
                    i += len(w)
                    ins.sync_info = mybir.SyncInfo(
                        on_wait=[], on_update=list(si.on_update or []))
                i += 1


def _get_nc():
    if "nc" not in _NC_CACHE:
        nc = build_nc()
        _split_multi_waits(nc)   # HW path only; CoreSim rejects raw drains
        _NC_CACHE["nc"] = nc
    return _NC_CACHE["nc"]


def make_in_maps(inputs):
    sent = np.ascontiguousarray(np.asarray(inputs["sentences"], dtype=np.float32))
    W = np.ascontiguousarray(np.asarray(inputs["W"], dtype=np.float32))
    bb = np.ascontiguousarray(np.asarray(inputs["b"], dtype=np.float32))
    st = np.ascontiguousarray(np.asarray(inputs["start_transitions"], dtype=np.float32))
    en = np.ascontiguousarray(np.asarray(inputs["end_transitions"], dtype=np.float32))
    tr = np.ascontiguousarray(np.asarray(inputs["transitions"], dtype=np.float32))
    tinit = np.tile(tr.ravel(), (128, 1)).astype(np.float32)
    tinit[0::C, :] = np.tile(st, 4)[None, :]
    rc = np.zeros((128, 64), dtype=np.float32)
    rc[:, 0:4] = [4.0, 3.0, 2.0, 1.0]
    rc[:, 4:8] = [0.0, 1.0, 2.0, 3.0]
    rc[:, 8:24] = np.eye(4, dtype=np.float32).ravel()[None, :]
    rc[:, 24:28] = en[None, :]
    rc[:, 28:44] = tr.ravel()[None, :]
    rc[:, 44:60] = tinit
    rc[:, 60:64] = bb[None, :]
    identc = np.eye(128, dtype=np.float32)
    return [{
        "sentences": sent[c * BC:(c + 1) * BC],
        "W": W, "identc": identc, "rowconsts": rc,
    } for c in range(NCORES)]


def kernel(**inputs):
    nc = _get_nc()
    in_maps = make_in_maps(inputs)
    res = run_bass_kernel_spmd(nc, in_maps, core_ids=list(range(NCORES)))
    tags = np.concatenate([res.results[c]["tags"] for c in range(NCORES)], axis=0)
    return tags.astype(np.int32)


if __name__ == "__main__":
    import reference
    inputs = {k: np.asarray(v) for k, v in reference.setup_inputs().items()}
    out = kernel(**inputs)
    print(out.shape, out.dtype, out[:2, :16])



# revision 5
# speedup vs baseline: 1.6735x; 1.6735x over previous
"""BERT-CRF Viterbi decode kernel for Trainium2 (Bass/Tile), 8-core data parallel.

Full inputs in, full outputs out. Internally shards batch B=64 across 8 cores
(8 sequences each). Per core, scan rows r = b*16 + c (c = chunk of 32 steps):

  Stage A (u-grouped): one DMA per group of 4 steps (12KB contiguous per
  partition row), PE transposes h-chunks, batched fp32r matmul (W^T x sT)
  -> emissions^T in PSUM, bias folded into the ACT PSUM->SBUF copy,
  fix-transpose back to [rows, 4] landing in the emissions scan tile.
  Groups processed in order 0,7,1,6,2,5,3,4 so both scans below progress.

  Fused under stage A, two within-chunk scans in (max,+) algebra:
    fwd (DVE):  red_u[i,j]  = max_k(red_{u-1}[i,k] + e_{u-1}[k] + trans[k,j])
                (chunk-start tag i -> tag j at u, emissions e_0..e_{u-1})
    bwd (Pool): B_u[x,i]    = max_j(trans[i,j] + e_{u+1}[j] + B_{u+1}[x,j])
                (tag i at u -> chunk-end tag x, emissions e_{u+1}..e_{L-1})
  Each step is 2 ops (TT add + reduce_max) using per-group precomputed
  G_u[k,j] = e_u[k]+trans[k,j] (fwd) and G'_u[i,j] = trans[i,j]+e_u[j] (bwd).

  Tail: chunk-boundary chains (fwd scores on DVE, bwd suffix scores on Pool,
  concurrent), broadcast to rows, then tags for ALL timesteps at once:
    tag_u = first-argmax_j( max_i(sb[i]+red_u[i,j]) + e_u[j]
                            + max_x(B_u[x,j]+tb[x]) )
  via a handful of big [128, L*4] vector ops. No backpointer storage, no
  one-hot composition chains.
"""
import sys
for p in ("/opt/trn_rl_repo", "/root/.axon_site/_ro/trn_rl_repo"):
    if p not in sys.path:
        sys.path.append(p)

import numpy as np
import concourse.bass as bass
import concourse.tile as tile
from concourse import mybir
from concourse.bass_utils import run_bass_kernel_spmd

F32 = mybir.dt.float32
F32R = mybir.dt.float32r
I32 = mybir.dt.int32
AX = mybir.AxisListType
OP = mybir.AluOpType
AF = mybir.ActivationFunctionType

B, T, H, K = 64, 512, 768, 4
NCORES = 8
BC = B // NCORES          # 8 sequences per core
C, L = 16, 32             # chunks per sequence, steps per chunk
ROWS = BC * C             # 128 partition rows
HCH = H // 128            # 6 h-chunks
UG = 4                    # steps per u-group (batched matmul width 4*128=512)
NG = L // UG              # 8 groups
PORDER = [0, 7, 1, 6, 2, 5, 3, 4]
NEG = -1.0e9

_NC_CACHE = {}


def build_nc():
    nc = bass.Bass()
    sent = nc.declare_dram_parameter("sentences", [BC, T, H], F32, isOutput=False)
    Wd = nc.declare_dram_parameter("W", [K, H], F32, isOutput=False)
    identd = nc.declare_dram_parameter("identc", [128, 128], F32, isOutput=False)
    # rowconsts[128, 64]: wfirst | biascol | binit | end | ttr | tinit
    rcd = nc.declare_dram_parameter("rowconsts", [128, 64], F32, isOutput=False)
    tagsd = nc.declare_dram_parameter("tags", [BC, T], I32, isOutput=True)

    with tile.TileContext(nc) as tc:
        with tc.tile_pool(name="singles", bufs=1) as singles, \
             tc.tile_pool(name="sent_pool", bufs=3) as sent_pool, \
             tc.tile_pool(name="st_pool", bufs=2) as st_pool, \
             tc.tile_pool(name="dve_tmp", bufs=3) as dve_tmp, \
             tc.tile_pool(name="pl_tmp", bufs=3) as pl_tmp, \
             tc.tile_pool(name="ps_tr", bufs=3, space="PSUM") as ps_tr, \
             tc.tile_pool(name="ps_eT", bufs=2, space="PSUM") as ps_eT, \
             tc.tile_pool(name="ps_fix", bufs=2, space="PSUM") as ps_fix:

            # ---------- prefetch first group's sentences ----------
            pre_sg = sent_pool.tile([128, UG, H], F32, tag="sent")
            g0 = PORDER[0]
            nc.sync.dma_start(pre_sg, bass.AP(
                tensor=sent[:].tensor, offset=g0 * UG * H,
                ap=[[T * H, BC], [L * H, C], [H, UG], [1, H]]))

            # ---------- constants ----------
            ident = singles.tile([128, 128], F32)
            nc.sync.dma_start(ident, identd[:])
            rc = singles.tile([128, 64], F32)
            nc.sync.dma_start(rc, rcd[:])
            wfirst = rc[:, 0:4]
            biascol = rc[0:K, 4:5]
            binit_xi = rc[:, 8:24].rearrange("p (x i) -> p x i", x=4)
            end8 = rc[0:BC, 24:28]
            ttr = rc[:, 28:44]
            ttr_ij = ttr.rearrange("p (i j) -> p i j", i=4)
            tinit_ij = rc[:, 44:60].rearrange("p (i j) -> p i j", i=4)

            # ---------- W^T in SBUF: wt[p = h within chunk, ch, k] ----------
            w_raw = singles.tile([K, H], F32)
            nc.sync.dma_start(w_raw, Wd[:])
            wt_sb = singles.tile([128, HCH, K], F32R)
            for ch in range(HCH):
                wt_ps = ps_fix.tile([128, K], F32, tag="fix")
                nc.tensor.transpose(wt_ps, w_raw[:, ch * 128:(ch + 1) * 128],
                                    ident[0:K, 0:K])
                nc.scalar.copy(wt_sb[:, ch, :], wt_ps)

            # scan emissions tile (bias included), written by stage A
            emsc = singles.tile([128, L * K], F32)
            emv = emsc.rearrange("p (u j) -> p u j", u=L)

            # scan state/aux tiles
            G_all = singles.tile([128, L, 4, 4], F32)    # G_u[k,j]
            Gp_all = singles.tile([128, L, 4, 4], F32)   # G'_u[i,j]
            red_all = singles.tile([128, L, 4, 4], F32)  # red_u[i,j]
            B_all = singles.tile([128, L, 4, 4], F32)    # B_u[x,i]

            # ---------- Stage A: DMA + PE + ACT streams ----------
            sA = nc.named_scope("stageA")
            sA.__enter__()
            for pos, g in enumerate(PORDER):
                if pos == 0:
                    sg = pre_sg
                else:
                    sg = sent_pool.tile([128, UG, H], F32, tag="sent")
                    nc.sync.dma_start(sg, bass.AP(
                        tensor=sent[:].tensor, offset=g * UG * H,
                        ap=[[T * H, BC], [L * H, C], [H, UG], [1, H]]))
                sT_sb = st_pool.tile([128, HCH, UG * 128], F32R, tag="sT")
                for ch in range(HCH):
                    trp = ps_tr.tile([128, UG * 128], F32, tag="trps")
                    for uu in range(UG):
                        nc.tensor.transpose(
                            trp[:, uu * 128:(uu + 1) * 128],
                            sg[:, uu, ch * 128:(ch + 1) * 128],
                            ident)
                    nc.scalar.copy(sT_sb[:, ch, :], trp)
                eT_ps = ps_eT.tile([4, UG * 128], F32, tag="eT")
                for ch in range(HCH):
                    nc.tensor.matmul(
                        eT_ps, wt_sb[:, ch, :], sT_sb[:, ch, :],
                        start=(ch == 0), stop=(ch == HCH - 1))
                # PSUM -> SBUF with bias folded in (b[k] per partition k)
                eT_sb = st_pool.tile([4, UG * 128], F32, tag="eTsb")
                nc.scalar.activation(eT_sb, eT_ps, AF.Identity, bias=biascol)
                for uu in range(UG):
                    u = g * UG + uu
                    fx = ps_fix.tile([128, K], F32, tag="fix")
                    nc.tensor.transpose(
                        fx, eT_sb[:, uu * 128:(uu + 1) * 128], ident[0:K, 0:K])
                    nc.scalar.copy(emsc[:, u * 4:(u + 1) * 4], fx)
            sA.__exit__(None, None, None)

            # ---------- Pool stream: G/G' precompute per group ----------
            # G_u[k,j] = e_u[k] + trans[k,j]; G'_u[i,j] = trans[i,j] + e_u[j]
            sG = nc.named_scope("gops")
            sG.__enter__()
            for g in PORDER:
                nc.gpsimd.tensor_tensor(
                    G_all[:, g * UG:(g + 1) * UG],
                    emv[:, g * UG:(g + 1) * UG, :].unsqueeze(3)
                        .to_broadcast((128, UG, 4, 4)),
                    ttr_ij.unsqueeze(1).to_broadcast((128, UG, 4, 4)),
                    OP.add)
                nc.gpsimd.tensor_tensor(
                    Gp_all[:, g * UG:(g + 1) * UG],
                    ttr_ij.unsqueeze(1).to_broadcast((128, UG, 4, 4)),
                    emv[:, g * UG:(g + 1) * UG, :].unsqueeze(2)
                        .to_broadcast((128, UG, 4, 4)),
                    OP.add)
            sG.__exit__(None, None, None)

            # ---------- DVE stream: fwd+bwd scans, wavefront order ----------
            # Wave w runs fwd step u=w (needs G[w-1], group (w-1)//4) and bwd
            # step u=31-w (needs G'[32-w], group (32-w)//4): consumes groups
            # in exactly PORDER so the scans track stage A.
            sF = nc.named_scope("scan")
            sF.__enter__()
            nc.vector.tensor_copy(red_all[:, 0], tinit_ij)
            nc.vector.tensor_copy(B_all[:, L - 1], binit_xi)
            for w in range(1, L):
                u = w
                ftmp = dve_tmp.tile([128, 4, 4, 4], F32, tag="ftmp")
                # cand[i,j,k] = red_{u-1}[i,k] + G_{u-1}[k,j]
                nc.vector.tensor_tensor(
                    ftmp,
                    red_all[:, u - 1].unsqueeze(2).to_broadcast((128, 4, 4, 4)),
                    G_all[:, u - 1].transpose([0, 2, 1]).unsqueeze(1)
                        .to_broadcast((128, 4, 4, 4)),
                    OP.add)
                nc.vector.reduce_max(red_all[:, u], ftmp, axis=AX.X)
                u = L - 1 - w
                btmp = dve_tmp.tile([128, 4, 4, 4], F32, tag="btmp")
                # cand[x,i,j] = B_{u+1}[x,j] + G'_{u+1}[i,j]
                nc.vector.tensor_tensor(
                    btmp,
                    B_all[:, u + 1].unsqueeze(2).to_broadcast((128, 4, 4, 4)),
                    Gp_all[:, u + 1].unsqueeze(1).to_broadcast((128, 4, 4, 4)),
                    OP.add)
                nc.vector.reduce_max(B_all[:, u], btmp, axis=AX.X)
            sF.__exit__(None, None, None)

            # ---------- chunk matrices to by-b layout ----------
            sP2 = nc.named_scope("p2")
            sP2.__enter__()
            Ac = singles.tile([128, 16], F32)
            # Ac[i,j] = red_{L-1}[i,j] + e_{L-1}[j]
            nc.vector.tensor_tensor(
                Ac.rearrange("p (i j) -> p i j", i=4),
                red_all[:, L - 1],
                emv[:, L - 1, :].unsqueeze(1).to_broadcast((128, 4, 4)),
                OP.add)
            abyb = singles.tile([BC, C * 16], F32)
            nc.sync.dma_start(abyb, Ac)
            abv = abyb.rearrange("p (c i j) -> p c i j", c=C, i=4)

            # ---------- fwd boundary chain (DVE): sb_c per chunk ----------
            sbound = singles.tile([BC, C * 4], F32)
            sbv = sbound.rearrange("p (c j) -> p c j", c=C)
            nc.vector.memset(sbound[:, 0:4], 0.0)
            for c in range(C - 1):
                p2tmp = dve_tmp.tile([BC, 4, 4], F32, tag="p2tmp")
                # tmp[j,i] = sb_c[i] + Ac_c[i,j]
                nc.vector.tensor_tensor(
                    p2tmp,
                    sbv[:, c, :].unsqueeze(1).to_broadcast((BC, 4, 4)),
                    abv[:, c].transpose([0, 2, 1]),
                    OP.add)
                nc.vector.reduce_max(sbv[:, c + 1, :], p2tmp, axis=AX.X)
            sbc = singles.tile([128, 4], F32)
            nc.sync.dma_start(sbc, sbound)
            sP2.__exit__(None, None, None)

            # ---------- bwd boundary chain (DVE): tb_c per chunk ----------
            sTB = nc.named_scope("tb")
            sTB.__enter__()
            tbound = singles.tile([BC, C * 4], F32)
            tbv = tbound.rearrange("p (c j) -> p c j", c=C)
            nc.vector.tensor_copy(tbv[:, C - 1, :], end8)
            for c in range(C - 2, -1, -1):
                ttmp = dve_tmp.tile([BC, 4, 4], F32, tag="ttmp")
                # tmp[x,j] = Ac_{c+1}[x,j] + tb_{c+1}[j]
                nc.vector.tensor_tensor(
                    ttmp,
                    abv[:, c + 1],
                    tbv[:, c + 1, :].unsqueeze(1).to_broadcast((BC, 4, 4)),
                    OP.add)
                nc.vector.reduce_max(tbv[:, c, :], ttmp, axis=AX.X)
            tbc = singles.tile([128, 4], F32)
            nc.sync.dma_start(tbc, tbound)
            sTB.__exit__(None, None, None)

            # ---------- combine: tags for all u at once ----------
            sCB = nc.named_scope("comb")
            sCB.__enter__()
            # Q_u[j] = max_x(B_u[x,j] + tb[x])   (TT on Pool, reduce on DVE)
            candQ = singles.tile([128, L, 4, 4], F32)
            nc.gpsimd.tensor_tensor(
                candQ,
                B_all.transpose([0, 1, 3, 2]),
                tbc.unsqueeze(1).unsqueeze(1).to_broadcast((128, L, 4, 4)),
                OP.add)
            Q = singles.tile([128, L, 4], F32)
            nc.vector.reduce_max(Q, candQ, axis=AX.X)

            # P_u[j] = max_i(sb[i] + red_u[i,j]) + e_u[j]   (DVE)
            candP = singles.tile([128, L, 4, 4], F32)
            nc.vector.tensor_tensor(
                candP,
                red_all.transpose([0, 1, 3, 2]),
                sbc.unsqueeze(1).unsqueeze(1).to_broadcast((128, L, 4, 4)),
                OP.add)
            P = singles.tile([128, L, 4], F32)
            nc.vector.reduce_max(P, candP, axis=AX.X)
            R = singles.tile([128, L, 4], F32)
            nc.vector.tensor_tensor(R, P, emv, OP.add)
            nc.vector.tensor_tensor(R, R, Q, OP.add)
            M = singles.tile([128, L], F32)
            nc.vector.reduce_max(M, R, axis=AX.X)
            eq = singles.tile([128, L, 4], F32)
            nc.vector.tensor_tensor(
                eq, R, M.unsqueeze(2).to_broadcast((128, L, 4)), OP.is_equal)
            nc.vector.tensor_tensor(
                eq, eq, wfirst.unsqueeze(1).to_broadcast((128, L, 4)), OP.mult)
            Wm = singles.tile([128, L], F32)
            nc.vector.reduce_max(Wm, eq, axis=AX.X)
            tagf = singles.tile([128, L], F32)
            nc.vector.tensor_scalar(tagf, Wm, -1.0, 4.0, OP.mult, OP.add)
            tagi = singles.tile([128, L], I32)
            nc.vector.tensor_copy(tagi, tagf)
            nc.sync.dma_start(tagsd[:].rearrange("b (c t) -> b c t", c=C), tagi)
            sCB.__exit__(None, None, None)

    return nc


def _split_multi_waits(nc):
    """Walrus (bass2jax path) allows very few embedded sync waits per
    instruction (PE matmul: exactly 1). Hoist multi-waits onto standalone
    single-wait InstDrain instructions on the same engine, preserving order."""
    for f in nc.m.functions:
        for blk in f.blocks:
            insts = blk.instructions
            i = 0
            while i < len(insts):
                ins = insts[i]
                si = ins.sync_info
                w = list(si.on_wait) if (si is not None and si.on_wait) else []
                if len(w) >= 2:
                    for k, wait in enumerate(w):
                        d = mybir.InstEventSemaphore(
                            name=nc.get_next_instruction_name(), ins=[], outs=[])
                        d.engine = ins.engine
                        d.sync_info = mybir.SyncInfo(on_wait=[wait], on_update=[])
                        insts.insert(i + k, d)
                    i += len(w)
                    ins.sync_info = mybir.SyncInfo(
                        on_wait=[], on_update=list(si.on_update or []))
                i += 1


def _get_nc():
    if "nc" not in _NC_CACHE:
        nc = build_nc()
        _split_multi_waits(nc)   # HW path only; CoreSim rejects raw drains
        _NC_CACHE["nc"] = nc
    return _NC_CACHE["nc"]


def make_in_maps(inputs):
    sent = np.ascontiguousarray(np.asarray(inputs["sentences"], dtype=np.float32))
    W = np.ascontiguousarray(np.asarray(inputs["W"], dtype=np.float32))
    bb = np.ascontiguousarray(np.asarray(inputs["b"], dtype=np.float32))
    st = np.ascontiguousarray(np.asarray(inputs["start_transitions"], dtype=np.float32))
    en = np.ascontiguousarray(np.asarray(inputs["end_transitions"], dtype=np.float32))
    tr = np.ascontiguousarray(np.asarray(inputs["transitions"], dtype=np.float32))
    tinit = np.tile(tr.ravel(), (128, 1)).astype(np.float32)
    tinit[0::C, :] = np.tile(st, 4)[None, :]
    binit = np.full((4, 4), NEG, dtype=np.float32)
    np.fill_diagonal(binit, 0.0)
    rc = np.zeros((128, 64), dtype=np.float32)
    rc[:, 0:4] = [4.0, 3.0, 2.0, 1.0]
    rc[0:K, 4] = bb
    rc[:, 8:24] = binit.ravel()[None, :]
    rc[:, 24:28] = en[None, :]
    rc[:, 28:44] = tr.ravel()[None, :]
    rc[:, 44:60] = tinit
    identc = np.eye(128, dtype=np.float32)
    return [{
        "sentences": sent[c * BC:(c + 1) * BC],
        "W": W, "identc": identc, "rowconsts": rc,
    } for c in range(NCORES)]


def kernel(**inputs):
    nc = _get_nc()
    in_maps = make_in_maps(inputs)
    res = run_bass_kernel_spmd(nc, in_maps, core_ids=list(range(NCORES)))
    tags = np.concatenate([res.results[c]["tags"] for c in range(NCORES)], axis=0)
    return tags.astype(np.int32)


if __name__ == "__main__":
    import reference
    inputs = {k: np.asarray(v) for k, v in reference.setup_inputs().items()}
    out = kernel(**inputs)
    print(out.shape, out.dtype, out[:2, :16])


# revision 16
# speedup vs baseline: 1.6876x; 1.0084x over previous
"""BERT-CRF Viterbi decode kernel for Trainium2 (Bass/Tile), 8-core data parallel.

Full inputs in, full outputs out. Internally shards batch B=64 across 8 cores
(8 sequences each). Per core, scan rows r = b*16 + c (c = chunk of 32 steps):

  Stage A (u-grouped): one DMA per group of 4 steps (12KB contiguous per
  partition row), PE transposes h-chunks, batched fp32r matmul (W^T x sT)
  -> emissions^T in PSUM, bias folded into the ACT PSUM->SBUF copy,
  fix-transpose back to [rows, 4] landing in the emissions scan tile.
  Groups processed in order 0,7,1,6,2,5,3,4 so both scans below progress.

  Fused under stage A, two within-chunk scans in (max,+) algebra:
    fwd (DVE):  red_u[i,j]  = max_k(red_{u-1}[i,k] + e_{u-1}[k] + trans[k,j])
                (chunk-start tag i -> tag j at u, emissions e_0..e_{u-1})
    bwd (Pool): B_u[x,i]    = max_j(trans[i,j] + e_{u+1}[j] + B_{u+1}[x,j])
                (tag i at u -> chunk-end tag x, emissions e_{u+1}..e_{L-1})
  Each step is 2 ops (TT add + reduce_max) using per-group precomputed
  G_u[k,j] = e_u[k]+trans[k,j] (fwd) and G'_u[i,j] = trans[i,j]+e_u[j] (bwd).

  Tail: chunk-boundary chains (fwd scores on DVE, bwd suffix scores on Pool,
  concurrent), broadcast to rows, then tags for ALL timesteps at once:
    tag_u = first-argmax_j( max_i(sb[i]+red_u[i,j]) + e_u[j]
                            + max_x(B_u[x,j]+tb[x]) )
  via a handful of big [128, L*4] vector ops. No backpointer storage, no
  one-hot composition chains.
"""
import sys
for p in ("/opt/trn_rl_repo", "/root/.axon_site/_ro/trn_rl_repo"):
    if p not in sys.path:
        sys.path.append(p)

import numpy as np
import concourse.bass as bass
import concourse.tile as tile
from concourse import mybir
from concourse.bass_utils import run_bass_kernel_spmd

F32 = mybir.dt.float32
F32R = mybir.dt.float32r
I32 = mybir.dt.int32
AX = mybir.AxisListType
OP = mybir.AluOpType
AF = mybir.ActivationFunctionType

B, T, H, K = 64, 512, 768, 4
NCORES = 8
BC = B // NCORES          # 8 sequences per core
C, L = 16, 32             # chunks per sequence, steps per chunk
ROWS = BC * C             # 128 partition rows
HCH = H // 128            # 6 h-chunks
UG = 4                    # steps per u-group (batched matmul width 4*128=512)
NG = L // UG              # 8 groups
PORDER = [0, 7, 1, 6, 2, 5, 3, 4]
NEG = -1.0e9

_NC_CACHE = {}


def build_nc():
    nc = bass.Bass()
    sent = nc.declare_dram_parameter("sentences", [BC, T, H], F32, isOutput=False)
    Wd = nc.declare_dram_parameter("W", [K, H], F32, isOutput=False)
    identd = nc.declare_dram_parameter("identc", [128, 128], F32, isOutput=False)
    # rowconsts[128, 64]: wfirst | biascol | binit | end | ttr | tinit
    rcd = nc.declare_dram_parameter("rowconsts", [128, 64], F32, isOutput=False)
    tagsd = nc.declare_dram_parameter("tags", [BC, T], I32, isOutput=True)

    with tile.TileContext(nc) as tc:
        with tc.tile_pool(name="singles", bufs=1) as singles, \
             tc.tile_pool(name="sent_pool", bufs=3) as sent_pool, \
             tc.tile_pool(name="st_pool", bufs=2) as st_pool, \
             tc.tile_pool(name="dve_tmp", bufs=3) as dve_tmp, \
             tc.tile_pool(name="pl_tmp", bufs=3) as pl_tmp, \
             tc.tile_pool(name="ps_tr", bufs=3, space="PSUM") as ps_tr, \
             tc.tile_pool(name="ps_eT", bufs=2, space="PSUM") as ps_eT, \
             tc.tile_pool(name="ps_fix", bufs=2, space="PSUM") as ps_fix:

            # ---------- constants (tiny DMAs first so PE/scans start early) ----------
            ident = singles.tile([128, 128], F32)
            nc.sync.dma_start(ident, identd[:])
            rc = singles.tile([128, 64], F32)
            nc.sync.dma_start(rc, rcd[:])

            # ---------- prefetch first group's sentences ----------
            pre_sg = sent_pool.tile([128, UG, H], F32, tag="sent")
            g0 = PORDER[0]
            nc.sync.dma_start(pre_sg, bass.AP(
                tensor=sent[:].tensor, offset=g0 * UG * H,
                ap=[[T * H, BC], [L * H, C], [H, UG], [1, H]]))
            wfirst = rc[:, 0:4]
            biascol = rc[0:K, 4:5]
            binit_xi = rc[:, 8:24].rearrange("p (x i) -> p x i", x=4)
            end8 = rc[0:BC, 24:28]
            ttr = rc[:, 28:44]
            ttr_ij = ttr.rearrange("p (i j) -> p i j", i=4)
            tinit_ij = rc[:, 44:60].rearrange("p (i j) -> p i j", i=4)
            ident4 = rc[0:K, 60:64]

            # ---------- W^T in SBUF: wt[p = h within chunk, ch, k] ----------
            w_raw = singles.tile([K, H], F32)
            nc.sync.dma_start(w_raw, Wd[:])
            wt_sb = singles.tile([128, HCH, K], F32R)
            for ch in range(HCH):
                wt_ps = ps_fix.tile([128, K], F32, tag="fix")
                nc.tensor.transpose(wt_ps, w_raw[:, ch * 128:(ch + 1) * 128],
                                    ident4)
                nc.scalar.copy(wt_sb[:, ch, :], wt_ps)

            # scan emissions tile (bias included), written by stage A
            emsc = singles.tile([128, L * K], F32)
            emv = emsc.rearrange("p (u j) -> p u j", u=L)

            # scan state/aux tiles
            G_all = singles.tile([128, L, 4, 4], F32)    # G_u[k,j]
            Gp_all = singles.tile([128, L, 4, 4], F32)   # G'_u[i,j]
            red_all = singles.tile([128, L, 4, 4], F32)  # red_u[i,j]
            B_all = singles.tile([128, L, 4, 4], F32)    # B_u[x,i]

            # ---------- Stage A: DMA + PE + ACT streams ----------
            sA = nc.named_scope("stageA")
            sA.__enter__()
            for pos, g in enumerate(PORDER):
                if pos == 0:
                    sg = pre_sg
                else:
                    sg = sent_pool.tile([128, UG, H], F32, tag="sent")
                    nc.sync.dma_start(sg, bass.AP(
                        tensor=sent[:].tensor, offset=g * UG * H,
                        ap=[[T * H, BC], [L * H, C], [H, UG], [1, H]]))
                sT_sb = st_pool.tile([128, HCH, UG * 128], F32R, tag="sT")
                for ch in range(HCH):
                    trp = ps_tr.tile([128, UG * 128], F32, tag="trps")
                    for uu in range(UG):
                        nc.tensor.transpose(
                            trp[:, uu * 128:(uu + 1) * 128],
                            sg[:, uu, ch * 128:(ch + 1) * 128],
                            ident)
                    nc.scalar.copy(sT_sb[:, ch, :], trp)
                eT_ps = ps_eT.tile([4, UG * 128], F32, tag="eT")
                for ch in range(HCH):
                    nc.tensor.matmul(
                        eT_ps, wt_sb[:, ch, :], sT_sb[:, ch, :],
                        start=(ch == 0), stop=(ch == HCH - 1))
                # PSUM -> SBUF with bias folded in (b[k] per partition k)
                eT_sb = st_pool.tile([4, UG * 128], F32, tag="eTsb")
                nc.scalar.activation(eT_sb, eT_ps, AF.Identity, bias=biascol)
                fq = ps_fix.tile([128, UG * K], F32, tag="fix")
                for uu in range(UG):
                    nc.tensor.transpose(
                        fq[:, uu * K:(uu + 1) * K],
                        eT_sb[:, uu * 128:(uu + 1) * 128], ident4)
                nc.scalar.copy(emsc[:, g * UG * K:(g + 1) * UG * K], fq)
            sA.__exit__(None, None, None)

            # ---------- Pool stream: G/G' precompute per group ----------
            # G_u[k,j] = e_u[k] + trans[k,j]; G'_u[i,j] = trans[i,j] + e_u[j]
            sG = nc.named_scope("gops")
            sG.__enter__()
            for g in PORDER:
                nc.gpsimd.tensor_tensor(
                    G_all[:, g * UG:(g + 1) * UG],
                    emv[:, g * UG:(g + 1) * UG, :].unsqueeze(3)
                        .to_broadcast((128, UG, 4, 4)),
                    ttr_ij.unsqueeze(1).to_broadcast((128, UG, 4, 4)),
                    OP.add)
                nc.gpsimd.tensor_tensor(
                    Gp_all[:, g * UG:(g + 1) * UG],
                    ttr_ij.unsqueeze(1).to_broadcast((128, UG, 4, 4)),
                    emv[:, g * UG:(g + 1) * UG, :].unsqueeze(2)
                        .to_broadcast((128, UG, 4, 4)),
                    OP.add)
            sG.__exit__(None, None, None)

            # ---------- DVE stream: fwd+bwd scans, wavefront order ----------
            # Wave w runs fwd step u=w (needs G[w-1], group (w-1)//4) and bwd
            # step u=31-w (needs G'[32-w], group (32-w)//4): consumes groups
            # in exactly PORDER so the scans track stage A.
            sF = nc.named_scope("scan")
            sF.__enter__()
            nc.vector.tensor_copy(red_all[:, 0], tinit_ij)
            nc.vector.tensor_copy(B_all[:, L - 1], binit_xi)
            for w in range(1, L):
                u = w
                ftmp = dve_tmp.tile([128, 4, 4, 4], F32, tag="ftmp")
                # cand[i,j,k] = red_{u-1}[i,k] + G_{u-1}[k,j]
                nc.vector.tensor_tensor(
                    ftmp,
                    red_all[:, u - 1].unsqueeze(2).to_broadcast((128, 4, 4, 4)),
                    G_all[:, u - 1].transpose([0, 2, 1]).unsqueeze(1)
                        .to_broadcast((128, 4, 4, 4)),
                    OP.add)
                nc.vector.reduce_max(red_all[:, u], ftmp, axis=AX.X)
                u = L - 1 - w
                btmp = dve_tmp.tile([128, 4, 4, 4], F32, tag="btmp")
                # cand[x,i,j] = B_{u+1}[x,j] + G'_{u+1}[i,j]
                nc.vector.tensor_tensor(
                    btmp,
                    B_all[:, u + 1].unsqueeze(2).to_broadcast((128, 4, 4, 4)),
                    Gp_all[:, u + 1].unsqueeze(1).to_broadcast((128, 4, 4, 4)),
                    OP.add)
                nc.vector.reduce_max(B_all[:, u], btmp, axis=AX.X)
            sF.__exit__(None, None, None)

            # ---------- chunk matrices to by-b layout ----------
            sP2 = nc.named_scope("p2")
            sP2.__enter__()
            Ac = singles.tile([128, 16], F32)
            # Ac[i,j] = red_{L-1}[i,j] + e_{L-1}[j]
            nc.vector.tensor_tensor(
                Ac.rearrange("p (i j) -> p i j", i=4),
                red_all[:, L - 1],
                emv[:, L - 1, :].unsqueeze(1).to_broadcast((128, 4, 4)),
                OP.add)
            abyb = singles.tile([BC, C * 16], F32)
            nc.sync.dma_start(abyb, Ac)
            abv = abyb.rearrange("p (c i j) -> p c i j", c=C, i=4)

            # ---------- fwd boundary chain (DVE): sb_c per chunk ----------
            sbt = singles.tile([BC, 2 * C * 4], F32)
            sbv = sbt[:, 0:C * 4].rearrange("p (c j) -> p c j", c=C)
            tbv = sbt[:, C * 4:2 * C * 4].rearrange("p (c j) -> p c j", c=C)
            nc.vector.memset(sbt[:, 0:4], 0.0)
            for c in range(C - 1):
                p2tmp = dve_tmp.tile([BC, 4, 4], F32, tag="p2tmp")
                # tmp[j,i] = sb_c[i] + Ac_c[i,j]
                nc.vector.tensor_tensor(
                    p2tmp,
                    sbv[:, c, :].unsqueeze(1).to_broadcast((BC, 4, 4)),
                    abv[:, c].transpose([0, 2, 1]),
                    OP.add)
                nc.vector.reduce_max(sbv[:, c + 1, :], p2tmp, axis=AX.X)
            sP2.__exit__(None, None, None)

            # ---------- bwd boundary chain (DVE): tb_c per chunk ----------
            sTB = nc.named_scope("tb")
            sTB.__enter__()
            nc.vector.tensor_copy(tbv[:, C - 1, :], end8)
            for c in range(C - 2, -1, -1):
                ttmp = dve_tmp.tile([BC, 4, 4], F32, tag="ttmp")
                # tmp[x,j] = Ac_{c+1}[x,j] + tb_{c+1}[j]
                nc.vector.tensor_tensor(
                    ttmp,
                    abv[:, c + 1],
                    tbv[:, c + 1, :].unsqueeze(1).to_broadcast((BC, 4, 4)),
                    OP.add)
                nc.vector.reduce_max(tbv[:, c, :], ttmp, axis=AX.X)
            sbc = singles.tile([128, 4], F32)
            nc.sync.dma_start(sbc, sbt[:, 0:C * 4])
            tbc = singles.tile([128, 4], F32)
            nc.sync.dma_start(tbc, sbt[:, C * 4:2 * C * 4])
            sTB.__exit__(None, None, None)

            # ---------- combine: tags for all u at once ----------
            sCB = nc.named_scope("comb")
            sCB.__enter__()
            # Q_u[j] = max_x(B_u[x,j] + tb[x])   (TT on Pool, reduce on DVE)
            candQ = singles.tile([128, L, 4, 4], F32)
            nc.gpsimd.tensor_tensor(
                candQ,
                B_all.transpose([0, 1, 3, 2]),
                tbc.unsqueeze(1).unsqueeze(1).to_broadcast((128, L, 4, 4)),
                OP.add)
            Q = singles.tile([128, L, 4], F32)
            nc.vector.reduce_max(Q, candQ, axis=AX.X)

            # P_u[j] = max_i(sb[i] + red_u[i,j]) + e_u[j]   (DVE)
            candP = singles.tile([128, L, 4, 4], F32)
            nc.vector.tensor_tensor(
                candP,
                red_all.transpose([0, 1, 3, 2]),
                sbc.unsqueeze(1).unsqueeze(1).to_broadcast((128, L, 4, 4)),
                OP.add)
            P = singles.tile([128, L, 4], F32)
            nc.vector.reduce_max(P, candP, axis=AX.X)
            R = singles.tile([128, L, 4], F32)
            nc.vector.tensor_tensor(R, P, emv, OP.add)
            nc.vector.tensor_tensor(R, R, Q, OP.add)
            M = singles.tile([128, L], F32)
            nc.vector.reduce_max(M, R, axis=AX.X)
            eq = singles.tile([128, L, 4], F32)
            nc.vector.tensor_tensor(
                eq, R, M.unsqueeze(2).to_broadcast((128, L, 4)), OP.is_equal)
            nc.vector.tensor_tensor(
                eq, eq, wfirst.unsqueeze(1).to_broadcast((128, L, 4)), OP.mult)
            Wm = singles.tile([128, L], F32)
            nc.vector.reduce_max(Wm, eq, axis=AX.X)
            tagf = singles.tile([128, L], F32)
            nc.vector.tensor_scalar(tagf, Wm, -1.0, 4.0, OP.mult, OP.add)
            tagi = singles.tile([128, L], I32)
            nc.vector.tensor_copy(tagi, tagf)
            nc.sync.dma_start(tagsd[:].rearrange("b (c t) -> b c t", c=C), tagi)
            sCB.__exit__(None, None, None)

    return nc


def _split_multi_waits(nc):
    """Walrus (bass2jax path) allows very few embedded sync waits per
    instruction (PE matmul: exactly 1). Hoist multi-waits onto standalone
    single-wait InstDrain instructions on the same engine, preserving order."""
    for f in nc.m.functions:
        for blk in f.blocks:
            insts = blk.instructions
            i = 0
            while i < len(insts):
                ins = insts[i]
                si = ins.sync_info
                w = list(si.on_wait) if (si is not None and si.on_wait) else []
                if len(w) >= 2:
                    for k, wait in enumerate(w):
                        d = mybir.InstEventSemaphore(
                            name=nc.get_next_instruction_name(), ins=[], outs=[])
                        d.engine = ins.engine
                        d.sync_info = mybir.SyncInfo(on_wait=[wait], on_update=[])
                        insts.insert(i + k, d)
                    i += len(w)
                    ins.sync_info = mybir.SyncInfo(
                        on_wait=[], on_update=list(si.on_update or []))
                i += 1


def _get_nc():
    if "nc" not in _NC_CACHE:
        nc = build_nc()
        _split_multi_waits(nc)   # HW path only; CoreSim rejects raw drains
        _NC_CACHE["nc"] = nc
    return _NC_CACHE["nc"]


def make_in_maps(inputs):
    sent = np.ascontiguousarray(np.asarray(inputs["sentences"], dtype=np.float32))
    W = np.ascontiguousarray(np.asarray(inputs["W"], dtype=np.float32))
    bb = np.ascontiguousarray(np.asarray(inputs["b"], dtype=np.float32))
    st = np.ascontiguousarray(np.asarray(inputs["start_transitions"], dtype=np.float32))
    en = np.ascontiguousarray(np.asarray(inputs["end_transitions"], dtype=np.float32))
    tr = np.ascontiguousarray(np.asarray(inputs["transitions"], dtype=np.float32))
    tinit = np.tile(tr.ravel(), (128, 1)).astype(np.float32)
    tinit[0::C, :] = np.tile(st, 4)[None, :]
    binit = np.full((4, 4), NEG, dtype=np.float32)
    np.fill_diagonal(binit, 0.0)
    rc = np.zeros((128, 64), dtype=np.float32)
    rc[:, 0:4] = [4.0, 3.0, 2.0, 1.0]
    rc[0:K, 4] = bb
    rc[:, 8:24] = binit.ravel()[None, :]
    rc[:, 24:28] = en[None, :]
    rc[:, 28:44] = tr.ravel()[None, :]
    rc[:, 44:60] = tinit
    rc[0:K, 60:64] = np.eye(K, dtype=np.float32)
    identc = np.eye(128, dtype=np.float32)
    return [{
        "sentences": sent[c * BC:(c + 1) * BC],
        "W": W, "identc": identc, "rowconsts": rc,
    } for c in range(NCORES)]


def kernel(**inputs):
    nc = _get_nc()
    in_maps = make_in_maps(inputs)
    res = run_bass_kernel_spmd(nc, in_maps, core_ids=list(range(NCORES)))
    tags = np.concatenate([res.results[c]["tags"] for c in range(NCORES)], axis=0)
    return tags.astype(np.int32)


if __name__ == "__main__":
    import reference
    inputs = {k: np.asarray(v) for k, v in reference.setup_inputs().items()}
    out = kernel(**inputs)
    print(out.shape, out.dtype, out[:2, :16])


# revision 24
# speedup vs baseline: 1.8686x; 1.1073x over previous
"""BERT-CRF Viterbi decode kernel for Trainium2 (Bass/Tile), 8-core data parallel.

Full inputs in, full outputs out. Internally shards batch B=64 across 8 cores
(8 sequences each). Per core, scan rows r = b*16 + c (c = chunk of 32 steps):

  Stage A (u-grouped): one DMA per group of 4 steps (12KB contiguous per
  partition row), PE transposes h-chunks, batched fp32r matmul (W^T x sT)
  -> emissions^T in PSUM, bias folded into the ACT PSUM->SBUF copy,
  fix-transpose back to [rows, 4] landing in the emissions scan tile.
  Groups processed in order 0,7,1,6,2,5,3,4 so both scans below progress.

  Fused under stage A, two within-chunk scans in (max,+) algebra:
    fwd (DVE):  red_u[i,j]  = max_k(red_{u-1}[i,k] + e_{u-1}[k] + trans[k,j])
                (chunk-start tag i -> tag j at u, emissions e_0..e_{u-1})
    bwd (Pool): B_u[x,i]    = max_j(trans[i,j] + e_{u+1}[j] + B_{u+1}[x,j])
                (tag i at u -> chunk-end tag x, emissions e_{u+1}..e_{L-1})
  Each step is 2 ops (TT add + reduce_max) using per-group precomputed
  G_u[k,j] = e_u[k]+trans[k,j] (fwd) and G'_u[i,j] = trans[i,j]+e_u[j] (bwd).

  Tail: chunk-boundary chains (fwd scores on DVE, bwd suffix scores on Pool,
  concurrent), broadcast to rows, then tags for ALL timesteps at once:
    tag_u = first-argmax_j( max_i(sb[i]+red_u[i,j]) + e_u[j]
                            + max_x(B_u[x,j]+tb[x]) )
  via a handful of big [128, L*4] vector ops. No backpointer storage, no
  one-hot composition chains.
"""
import sys
for p in ("/opt/trn_rl_repo", "/root/.axon_site/_ro/trn_rl_repo"):
    if p not in sys.path:
        sys.path.append(p)

import numpy as np
import concourse.bass as bass
import concourse.tile as tile
from concourse import mybir
from concourse.bass_utils import run_bass_kernel_spmd

F32 = mybir.dt.float32
F32R = mybir.dt.float32r
I32 = mybir.dt.int32
AX = mybir.AxisListType
OP = mybir.AluOpType
AF = mybir.ActivationFunctionType

B, T, H, K = 64, 512, 768, 4
NCORES = 8
BC = B // NCORES          # 8 sequences per core
C, L = 16, 32             # chunks per sequence, steps per chunk
ROWS = BC * C             # 128 partition rows
HCH = H // 128            # 6 h-chunks
UG = 4                    # steps per u-group (batched matmul width 4*128=512)
NG = L // UG              # 8 groups
PORDER = [0, 7, 1, 6, 2, 5, 3, 4]
NEG = -1.0e9

_NC_CACHE = {}


def build_nc():
    nc = bass.Bass()
    sent = nc.declare_dram_parameter("sentences", [BC, T, H], F32, isOutput=False)
    Wd = nc.declare_dram_parameter("W", [K, H], F32, isOutput=False)
    identd = nc.declare_dram_parameter("identc", [128, 128], F32, isOutput=False)
    # rowconsts[128, 64]: wfirst | biascol | binit | end | ttr | tinit
    rcd = nc.declare_dram_parameter("rowconsts", [128, 64], F32, isOutput=False)
    tagsd = nc.declare_dram_parameter("tags", [BC, T], I32, isOutput=True)

    with tile.TileContext(nc) as tc:
        with tc.tile_pool(name="singles", bufs=1) as singles, \
             tc.tile_pool(name="sent_pool", bufs=3) as sent_pool, \
             tc.tile_pool(name="st_pool", bufs=2) as st_pool, \
             tc.tile_pool(name="dve_tmp", bufs=3) as dve_tmp, \
             tc.tile_pool(name="pl_tmp", bufs=3) as pl_tmp, \
             tc.tile_pool(name="ps_tr", bufs=3, space="PSUM") as ps_tr, \
             tc.tile_pool(name="ps_eT", bufs=2, space="PSUM") as ps_eT, \
             tc.tile_pool(name="ps_fix", bufs=2, space="PSUM") as ps_fix:

            # ---------- first step's sentences + constants (pipelined start) ----------
            # Group 0 is split into per-step DMAs so the first transpose can
            # start after ~400KB instead of ~1.6MB.
            pre_sg = sent_pool.tile([128, UG, H], F32, tag="sent")
            g0 = PORDER[0]
            for uu in range(UG):
                nc.sync.dma_start(pre_sg[:, uu, :], bass.AP(
                    tensor=sent[:].tensor, offset=(g0 * UG + uu) * H,
                    ap=[[T * H, BC], [L * H, C], [1, H]]))
                if uu == 0:
                    ident = singles.tile([128, 128], F32)
                    nc.sync.dma_start(ident, identd[:])
                    rc = singles.tile([128, 64], F32)
                    nc.sync.dma_start(rc, rcd[:])
            wfirst = rc[:, 0:4]
            biascol = rc[0:K, 4:5]
            binit_xi = rc[:, 8:24].rearrange("p (x i) -> p x i", x=4)
            end8 = rc[0:BC, 24:28]
            ttr = rc[:, 28:44]
            ttr_ij = ttr.rearrange("p (i j) -> p i j", i=4)
            tinit_ij = rc[:, 44:60].rearrange("p (i j) -> p i j", i=4)
            ident4 = rc[0:K, 60:64]

            # ---------- W^T in SBUF: wt[p = h within chunk, ch, k] ----------
            w_raw = singles.tile([K, H], F32)
            nc.sync.dma_start(w_raw, Wd[:])
            wt_sb = singles.tile([128, HCH, K], F32R)
            for ch in range(HCH):
                wt_ps = ps_fix.tile([128, UG * K], F32, tag="fq")
                nc.tensor.transpose(wt_ps[:, 0:K],
                                    w_raw[:, ch * 128:(ch + 1) * 128], ident4)
                nc.scalar.copy(wt_sb[:, ch, :], wt_ps[:, 0:K])

            # scan emissions tile (bias included), written by stage A
            emsc = singles.tile([128, L * K], F32)
            emv = emsc.rearrange("p (u j) -> p u j", u=L)

            # scan state/aux tiles
            G_all = singles.tile([128, L, 4, 4], F32)    # G_u[k,j]
            Gp_all = singles.tile([128, L, 4, 4], F32)   # G'_u[i,j]
            red_all = singles.tile([128, L, 4, 4], F32)  # red_u[i,j]
            B_all = singles.tile([128, L, 4, 4], F32)    # B_u[x,i]

            # ---------- Stage A: DMA + PE + ACT streams ----------
            sA = nc.named_scope("stageA")
            sA.__enter__()
            fq_tiles = {}
            for pos, g in enumerate(PORDER):
                if pos == 0:
                    sg = pre_sg
                else:
                    sg = sent_pool.tile([128, UG, H], F32, tag="sent")
                    nc.sync.dma_start(sg, bass.AP(
                        tensor=sent[:].tensor, offset=g * UG * H,
                        ap=[[T * H, BC], [L * H, C], [H, UG], [1, H]]))
                sT_sb = st_pool.tile([128, HCH, UG * 128], F32R, tag="sT")
                for ch in range(HCH):
                    trp = ps_tr.tile([128, UG * 128], F32, tag="trps")
                    for uu in range(UG):
                        nc.tensor.transpose(
                            trp[:, uu * 128:(uu + 1) * 128],
                            sg[:, uu, ch * 128:(ch + 1) * 128],
                            ident)
                    nc.scalar.copy(sT_sb[:, ch, :], trp)
                eT_ps = ps_eT.tile([4, UG * 128], F32, tag="eT")
                for ch in range(HCH):
                    nc.tensor.matmul(
                        eT_ps, wt_sb[:, ch, :], sT_sb[:, ch, :],
                        start=(ch == 0), stop=(ch == HCH - 1))
                # PSUM -> SBUF with bias folded in (b[k] per partition k)
                eT_sb = st_pool.tile([4, UG * 128], F32, tag="eTsb")
                nc.scalar.activation(eT_sb, eT_ps, AF.Identity, bias=biascol)
                fq = ps_fix.tile([128, UG * K], F32, tag="fq")
                for uu in range(UG):
                    nc.tensor.transpose(
                        fq[:, uu * K:(uu + 1) * K],
                        eT_sb[:, uu * 128:(uu + 1) * 128], ident4)
                fq_tiles[g] = fq
            sA.__exit__(None, None, None)

            # ---------- DVE stream: emsc copies + fwd scan ----------
            # fq copies emitted in PORDER arrival; fwd steps consume groups in
            # numeric order, interleaved so the DVE FIFO never stalls long.
            sF = nc.named_scope("scan")
            sF.__enter__()

            def emsc_copy(g):
                nc.vector.tensor_copy(
                    emsc[:, g * UG * K:(g + 1) * UG * K], fq_tiles[g])

            def fwd_step(u):
                ftmp = dve_tmp.tile([128, 4, 4, 4], F32, tag="ftmp")
                # cand[i,j,k] = red_{u-1}[i,k] + G_{u-1}[k,j]
                nc.vector.tensor_tensor(
                    ftmp,
                    red_all[:, u - 1].unsqueeze(2).to_broadcast((128, 4, 4, 4)),
                    G_all[:, u - 1].transpose([0, 2, 1]).unsqueeze(1)
                        .to_broadcast((128, 4, 4, 4)),
                    OP.add)
                nc.vector.reduce_max(red_all[:, u], ftmp, axis=AX.X)

            def bwd_step(u):
                btmp = dve_tmp.tile([128, 4, 4, 4], F32, tag="btmp")
                # cand[x,i,j] = B_{u+1}[x,j] + G'_{u+1}[i,j]
                nc.vector.tensor_tensor(
                    btmp,
                    B_all[:, u + 1].unsqueeze(2).to_broadcast((128, 4, 4, 4)),
                    Gp_all[:, u + 1].unsqueeze(1).to_broadcast((128, 4, 4, 4)),
                    OP.add)
                nc.vector.reduce_max(B_all[:, u], btmp, axis=AX.X)

            def waves(lo, hi):
                for w in range(lo, hi):
                    fwd_step(w)
                    bwd_step(L - 1 - w)

            nc.vector.tensor_copy(red_all[:, 0], tinit_ij)
            nc.vector.tensor_copy(B_all[:, L - 1], binit_xi)
            emsc_copy(0)
            emsc_copy(7)
            waves(1, 5)
            emsc_copy(1)
            emsc_copy(6)
            waves(5, 9)
            emsc_copy(2)
            emsc_copy(5)
            waves(9, 13)
            emsc_copy(3)
            emsc_copy(4)
            waves(13, L)
            sF.__exit__(None, None, None)

            # ---------- Pool stream: G/G' per group ----------
            # G_u[k,j] = e_u[k] + trans[k,j]; G'_u[i,j] = trans[i,j] + e_u[j]
            sB = nc.named_scope("gops")
            sB.__enter__()
            for g in PORDER:
                nc.gpsimd.tensor_tensor(
                    G_all[:, g * UG:(g + 1) * UG],
                    emv[:, g * UG:(g + 1) * UG, :].unsqueeze(3)
                        .to_broadcast((128, UG, 4, 4)),
                    ttr_ij.unsqueeze(1).to_broadcast((128, UG, 4, 4)),
                    OP.add)
                nc.gpsimd.tensor_tensor(
                    Gp_all[:, g * UG:(g + 1) * UG],
                    ttr_ij.unsqueeze(1).to_broadcast((128, UG, 4, 4)),
                    emv[:, g * UG:(g + 1) * UG, :].unsqueeze(2)
                        .to_broadcast((128, UG, 4, 4)),
                    OP.add)
            sB.__exit__(None, None, None)

            # ---------- chunk matrices to by-b layout ----------
            sP2 = nc.named_scope("p2")
            sP2.__enter__()
            Ac = singles.tile([128, 16], F32)
            # Ac[i,j] = red_{L-1}[i,j] + e_{L-1}[j]
            nc.vector.tensor_tensor(
                Ac.rearrange("p (i j) -> p i j", i=4),
                red_all[:, L - 1],
                emv[:, L - 1, :].unsqueeze(1).to_broadcast((128, 4, 4)),
                OP.add)
            abyb = singles.tile([BC, C * 16], F32)
            nc.sync.dma_start(abyb, Ac)
            abv = abyb.rearrange("p (c i j) -> p c i j", c=C, i=4)

            # ---------- fwd boundary chain (DVE): sb_c per chunk ----------
            sbt = singles.tile([BC, 2 * C * 4], F32)
            sbv = sbt[:, 0:C * 4].rearrange("p (c j) -> p c j", c=C)
            tbv = sbt[:, C * 4:2 * C * 4].rearrange("p (c j) -> p c j", c=C)
            nc.vector.memset(sbt[:, 0:4], 0.0)
            for c in range(C - 1):
                p2tmp = dve_tmp.tile([BC, 4, 4], F32, tag="p2tmp")
                # tmp[j,i] = sb_c[i] + Ac_c[i,j]
                nc.vector.tensor_tensor(
                    p2tmp,
                    sbv[:, c, :].unsqueeze(1).to_broadcast((BC, 4, 4)),
                    abv[:, c].transpose([0, 2, 1]),
                    OP.add)
                nc.vector.reduce_max(sbv[:, c + 1, :], p2tmp, axis=AX.X)
            sP2.__exit__(None, None, None)

            # ---------- bwd boundary chain (DVE): tb_c per chunk ----------
            sTB = nc.named_scope("tb")
            sTB.__enter__()
            nc.vector.tensor_copy(tbv[:, C - 1, :], end8)
            for c in range(C - 2, -1, -1):
                ttmp = dve_tmp.tile([BC, 4, 4], F32, tag="ttmp")
                # tmp[x,j] = Ac_{c+1}[x,j] + tb_{c+1}[j]
                nc.vector.tensor_tensor(
                    ttmp,
                    abv[:, c + 1],
                    tbv[:, c + 1, :].unsqueeze(1).to_broadcast((BC, 4, 4)),
                    OP.add)
                nc.vector.reduce_max(tbv[:, c, :], ttmp, axis=AX.X)
            sbc = singles.tile([128, 4], F32)
            nc.sync.dma_start(sbc, sbt[:, 0:C * 4])
            tbc = singles.tile([128, 4], F32)
            nc.sync.dma_start(tbc, sbt[:, C * 4:2 * C * 4])
            sTB.__exit__(None, None, None)

            # ---------- combine: tags for all u at once ----------
            sCB = nc.named_scope("comb")
            sCB.__enter__()
            # Q_u[j] = max_x(B_u[x,j] + tb[x])   (TT on Pool, reduce on DVE)
            candQ = singles.tile([128, L, 4, 4], F32)
            nc.gpsimd.tensor_tensor(
                candQ,
                B_all.transpose([0, 1, 3, 2]),
                tbc.unsqueeze(1).unsqueeze(1).to_broadcast((128, L, 4, 4)),
                OP.add)
            Q = singles.tile([128, L, 4], F32)
            nc.vector.reduce_max(Q, candQ, axis=AX.X)

            # P_u[j] = max_i(sb[i] + red_u[i,j]) + e_u[j]   (DVE)
            candP = singles.tile([128, L, 4, 4], F32)
            nc.vector.tensor_tensor(
                candP,
                red_all.transpose([0, 1, 3, 2]),
                sbc.unsqueeze(1).unsqueeze(1).to_broadcast((128, L, 4, 4)),
                OP.add)
            P = singles.tile([128, L, 4], F32)
            nc.vector.reduce_max(P, candP, axis=AX.X)
            R = singles.tile([128, L, 4], F32)
            nc.vector.tensor_tensor(R, P, emv, OP.add)
            nc.vector.tensor_tensor(R, R, Q, OP.add)
            M = singles.tile([128, L], F32)
            nc.vector.reduce_max(M, R, axis=AX.X)
            eq = singles.tile([128, L, 4], F32)
            nc.vector.tensor_tensor(
                eq, R, M.unsqueeze(2).to_broadcast((128, L, 4)), OP.is_equal)
            nc.vector.tensor_tensor(
                eq, eq, wfirst.unsqueeze(1).to_broadcast((128, L, 4)), OP.mult)
            Wm = singles.tile([128, L], F32)
            nc.vector.reduce_max(Wm, eq, axis=AX.X)
            tagf = singles.tile([128, L], F32)
            nc.vector.tensor_scalar(tagf, Wm, -1.0, 4.0, OP.mult, OP.add)
            tagi = singles.tile([128, L], I32)
            nc.vector.tensor_copy(tagi, tagf)
            nc.sync.dma_start(tagsd[:].rearrange("b (c t) -> b c t", c=C), tagi)
            sCB.__exit__(None, None, None)

    return nc


def _split_multi_waits(nc):
    """Walrus (bass2jax path) allows very few embedded sync waits per
    instruction (PE matmul: exactly 1). Hoist multi-waits onto standalone
    single-wait InstDrain instructions on the same engine, preserving order."""
    for f in nc.m.functions:
        for blk in f.blocks:
            insts = blk.instructions
            i = 0
            while i < len(insts):
                ins = insts[i]
                si = ins.sync_info
                w = list(si.on_wait) if (si is not None and si.on_wait) else []
                if len(w) >= 2:
                    for k, wait in enumerate(w):
                        d = mybir.InstEventSemaphore(
                            name=nc.get_next_instruction_name(), ins=[], outs=[])
                        d.engine = ins.engine
                        d.sync_info = mybir.SyncInfo(on_wait=[wait], on_update=[])
                        insts.insert(i + k, d)
                    i += len(w)
                    ins.sync_info = mybir.SyncInfo(
                        on_wait=[], on_update=list(si.on_update or []))
                i += 1


def _get_nc():
    if "nc" not in _NC_CACHE:
        nc = build_nc()
        _split_multi_waits(nc)   # HW path only; CoreSim rejects raw drains
        _NC_CACHE["nc"] = nc
    return _NC_CACHE["nc"]


def make_in_maps(inputs):
    sent = np.ascontiguousarray(np.asarray(inputs["sentences"], dtype=np.float32))
    W = np.ascontiguousarray(np.asarray(inputs["W"], dtype=np.float32))
    bb = np.ascontiguousarray(np.asarray(inputs["b"], dtype=np.float32))
    st = np.ascontiguousarray(np.asarray(inputs["start_transitions"], dtype=np.float32))
    en = np.ascontiguousarray(np.asarray(inputs["end_transitions"], dtype=np.float32))
    tr = np.ascontiguousarray(np.asarray(inputs["transitions"], dtype=np.float32))
    tinit = np.tile(tr.ravel(), (128, 1)).astype(np.float32)
    tinit[0::C, :] = np.tile(st, 4)[None, :]
    binit = np.full((4, 4), NEG, dtype=np.float32)
    np.fill_diagonal(binit, 0.0)
    rc = np.zeros((128, 64), dtype=np.float32)
    rc[:, 0:4] = [4.0, 3.0, 2.0, 1.0]
    rc[0:K, 4] = bb
    rc[:, 8:24] = binit.ravel()[None, :]
    rc[:, 24:28] = en[None, :]
    rc[:, 28:44] = tr.ravel()[None, :]
    rc[:, 44:60] = tinit
    rc[0:K, 60:64] = np.eye(K, dtype=np.float32)
    identc = np.eye(128, dtype=np.float32)
    return [{
        "sentences": sent[c * BC:(c + 1) * BC],
        "W": W, "identc": identc, "rowconsts": rc,
    } for c in range(NCORES)]


def kernel(**inputs):
    nc = _get_nc()
    in_maps = make_in_maps(inputs)
    res = run_bass_kernel_spmd(nc, in_maps, core_ids=list(range(NCORES)))
    tags = np.concatenate([res.results[c]["tags"] for c in range(NCORES)], axis=0)
    return tags.astype(np.int32)


if __name__ == "__main__":
    import reference
    inputs = {k: np.asarray(v) for k, v in reference.setup_inputs().items()}
    out = kernel(**inputs)
    print(out.shape, out.dtype, out[:2, :16])
